# revision 1
# baseline (speedup 1.0000x reference)
"""nn_GATLayer Trainium2 kernel: 8-core SPMD Bass/Tile implementation.

kernel(**inputs) takes the FULL inputs (feat [100000,128] f32, W, attn_l,
attn_r, bias, src/dst [1600000] i32) and returns the FULL output
[100000, 4, 16] f32.

Strategy (dst-sharded, collective-free):
  Host: sort edges by dst, shard by destination node range (12544 nodes per
  core), group each core's edges by 128-node destination blocks, pad each
  block to a common number of 128-edge tiles. All float math runs on device.
  Device (per core, same SPMD program):
    Phase A: h_aug = feat @ [W | wl | wr] for ALL nodes (replicated), where
      el = feat@wl, er = feat@wr fold the attention dot-products into the
      projection matmul. h_aug rows [h(64) | el(4) | er(4)] f32 in DRAM.
    Phase B (For_i loop over groups of destination blocks; the loop
      back-edge resets semaphores so DMA counts stay within 16-bit):
      per destination block of 128 nodes, for each 128-edge tile:
      indirect-DMA gather h_aug[src] rows and er = h_aug[dst, 68:72];
      ex = exp(leaky_relu(el_src + er_dst));  msg = [h_src * ex | ex];
      one-hot mask[e,n] = (dst_row[e] == n) via iota compare;
      psum[128n, 68] += mask^T @ msg  (TensorE segment-sum, accumulating
      numerator and softmax denominator together);
      out_block = psum[:, :64] / max(psum[:, 64:68], eps) + bias.
  Softmax max-subtraction is skipped: alpha = ex/sum(ex) is shift-invariant
  and logits are bounded (~|10|) for this distribution, so f32 exp is safe.
"""

import numpy as np
from contextlib import ExitStack

import concourse.bass as bass
import concourse.tile as tile
from concourse import bacc, mybir
from concourse.bass import ds
from concourse.masks import make_identity
from concourse.bass_utils import run_bass_kernel_spmd

F32 = mybir.dt.float32
I32 = mybir.dt.int32

P = 128
N_CORES = 8
NB = 98                    # destination blocks per core
D_OUT = 16
HEADS = 4
HD = HEADS * D_OUT         # 64
HAUG = HD + 2 * HEADS      # 72

N_NODES = 100000
IN_DIM = 128

LAST_RESULTS = None        # BassKernelResults of the most recent run


def _balance_perm(dst, n_cores, nb):
    """Within-core node relabeling balancing per-block in-degree sums."""
    import heapq
    npc = nb * P
    npad = n_cores * npc
    deg = np.bincount(dst, minlength=npad)
    perm = np.empty(npad, dtype=np.int64)
    for c in range(n_cores):
        d = deg[c * npc:(c + 1) * npc]
        order = np.argsort(-d, kind="stable")
        heap = [(0, b) for b in range(nb)]
        heapq.heapify(heap)
        counts = [0] * nb
        pos = np.empty(npc, dtype=np.int64)
        for i in order:
            while True:
                s_, b = heapq.heappop(heap)
                if counts[b] < P:
                    break
            pos[i] = b * P + counts[b]
            counts[b] += 1
            if counts[b] < P:
                heapq.heappush(heap, (s_ + int(d[i]), b))
        perm[c * npc:(c + 1) * npc] = c * npc + pos
    return perm


def build_host_data(feat, src, dst, n_cores=N_CORES, nb=NB):
    npc = nb * P
    npad = n_cores * npc
    perm = _balance_perm(dst, n_cores, nb)
    src = perm[src]
    dst = perm[dst]
    order = np.argsort(dst, kind="stable")
    es = src[order]
    ed = dst[order]

    in_dim = feat.shape[1]
    featT = np.zeros((in_dim, npad), dtype=np.float32)
    featT[:, perm[:feat.shape[0]]] = np.ascontiguousarray(feat.T)

    core_lo = np.searchsorted(ed, np.arange(n_cores) * npc, side="left")
    core_hi = np.searchsorted(ed, (np.arange(n_cores) + 1) * npc, side="left")

    tmax = 1
    percore = []
    for c in range(n_cores):
        s, e = core_lo[c], core_hi[c]
        dloc = ed[s:e] - c * npc
        cnt = np.bincount(dloc // P, minlength=nb)
        tmax = max(tmax, int(np.ceil(cnt.max() / P)))
        percore.append((es[s:e], dloc, cnt))

    nt = nb * tmax
    srcs, rows, edsts = [], [], []
    for c in range(n_cores):
        e_src, dloc, cnt = percore[c]
        src_slot = np.zeros((nb, tmax * P), dtype=np.int32)
        row_slot = np.full((nb, tmax * P), 255.0, dtype=np.float32)
        dst_slot = np.zeros((nb, tmax * P), dtype=np.int32)
        off = 0
        for b in range(nb):
            k = cnt[b]
            src_slot[b, :k] = e_src[off:off + k]
            row_slot[b, :k] = (dloc[off:off + k] - b * P).astype(np.float32)
            dst_slot[b, :k] = dloc[off:off + k] + c * npc
            off += k
        srcs.append(np.ascontiguousarray(src_slot.reshape(nt, P).T))
        rows.append(np.ascontiguousarray(row_slot.reshape(nt, P).T))
        edsts.append(np.ascontiguousarray(dst_slot.reshape(nt, P).T))

    return dict(featT=featT, srcs=srcs, rows=rows, edsts=edsts, perm=perm,
                tmax=tmax, nt=nt, npc=npc, npad=npad)


def build_program(tmax, nb=NB, in_dim=IN_DIM, n_cores=N_CORES,
                  blk_group=49, den_eps=1e-6, repeat=1):
    npc = nb * P
    npad = n_cores * npc
    nt = nb * tmax
    ntile_proj = npad // P
    while nb % blk_group != 0:
        blk_group -= 1
    n_iter = nb // blk_group
    G = blk_group

    nc = bacc.Bacc("TRN2", target_bir_lowering=False, debug=False,
                   num_devices=n_cores)

    featT_d = nc.dram_tensor("featT", [in_dim, npad], F32, kind="ExternalInput")
    w_d = nc.dram_tensor("W", [in_dim, HD], F32, kind="ExternalInput")
    al_d = nc.dram_tensor("attn_l", [HEADS, D_OUT], F32, kind="ExternalInput")
    ar_d = nc.dram_tensor("attn_r", [HEADS, D_OUT], F32, kind="ExternalInput")
    bias_d = nc.dram_tensor("bias", [HD], F32, kind="ExternalInput")
    src_d = nc.dram_tensor("srcs", [P, nt], I32, kind="ExternalInput")
    row_d = nc.dram_tensor("rows", [P, nt], F32, kind="ExternalInput")

    hA_d = nc.dram_tensor("h_aug", [npad, HAUG], F32, kind="Internal")
    out_d = nc.dram_tensor("out", [npc, HD], F32, kind="ExternalOutput")

    with tile.TileContext(nc) as tc, ExitStack() as ctx:
        cpool = ctx.enter_context(tc.tile_pool(name="const", bufs=1))
        ppool = ctx.enter_context(tc.tile_pool(name="proj", bufs=4))
        pspool = ctx.enter_context(tc.tile_pool(name="psA", bufs=2, space="PSUM"))
        stpool = ctx.enter_context(tc.tile_pool(name="stage", bufs=2))
        epool = ctx.enter_context(tc.tile_pool(name="edge", bufs=3))
        spool = ctx.enter_context(tc.tile_pool(name="small", bufs=4))
        psB = ctx.enter_context(tc.tile_pool(name="psB", bufs=4, space="PSUM"))
        psR = ctx.enter_context(tc.tile_pool(name="psR", bufs=2, space="PSUM"))

        ones_row = cpool.tile([1, P], F32)
        nc.vector.memset(ones_row[:], 1.0)
        iota_i = cpool.tile([P, P], I32)
        nc.gpsimd.iota(iota_i[:], pattern=[[1, P]], base=0, channel_multiplier=0)
        iota_f = cpool.tile([P, P], F32)
        nc.vector.tensor_copy(iota_f[:], iota_i[:])
        ident = cpool.tile([P, P], F32)
        make_identity(nc, ident[:])

        def pe_broadcast(row_ap, width):
            ps = pspool.tile([P, width], F32)
            nc.tensor.matmul(out=ps[:], lhsT=ones_row[:], rhs=row_ap,
                             start=True, stop=True)
            t = cpool.tile([P, width], F32)
            nc.vector.tensor_copy(t[:], ps[:])
            return t

        al_row = cpool.tile([1, HD], F32)
        nc.sync.dma_start(al_row[:], al_d[:].rearrange("h d -> (h d)").unsqueeze(0))
        ar_row = cpool.tile([1, HD], F32)
        nc.sync.dma_start(ar_row[:], ar_d[:].rearrange("h d -> (h d)").unsqueeze(0))
        b_row = cpool.tile([1, HD], F32)
        nc.sync.dma_start(b_row[:], bias_d[:].unsqueeze(0))

        al_b = pe_broadcast(al_row[:], HD)
        ar_b = pe_broadcast(ar_row[:], HD)
        bias_b = pe_broadcast(b_row[:], HD)

        w_aug = cpool.tile([P, HAUG], F32)
        nc.sync.dma_start(w_aug[:, 0:HD], w_d[:, :])
        tmp = cpool.tile([P, HD], F32)
        nc.vector.tensor_tensor(out=tmp[:], in0=w_aug[:, 0:HD], in1=al_b[:],
                                op=mybir.AluOpType.mult)
        nc.vector.tensor_reduce(
            out=w_aug[:, HD:HD + HEADS],
            in_=tmp[:].rearrange("p (h d) -> p h d", d=D_OUT),
            axis=mybir.AxisListType.X, op=mybir.AluOpType.add)
        tmp2 = cpool.tile([P, HD], F32)
        nc.vector.tensor_tensor(out=tmp2[:], in0=w_aug[:, 0:HD], in1=ar_b[:],
                                op=mybir.AluOpType.mult)
        nc.vector.tensor_reduce(
            out=w_aug[:, HD + HEADS:HAUG],
            in_=tmp2[:].rearrange("p (h d) -> p h d", d=D_OUT),
            axis=mybir.AxisListType.X, op=mybir.AluOpType.add)

        # ---- Phase A: projection (replicated over all nodes) ----
        PROJ_G = 8
        assert ntile_proj % PROJ_G == 0
        for i in range(ntile_proj // PROJ_G):
            ft = ppool.tile([P, PROJ_G * P], F32)
            nc.sync.dma_start(ft[:], featT_d[:, i * PROJ_G * P:(i + 1) * PROJ_G * P])
            hb = ppool.tile([P, PROJ_G * HAUG], F32)
            for j in range(PROJ_G):
                ps = pspool.tile([P, HAUG], F32)
                nc.tensor.matmul(out=ps[:], lhsT=ft[:, j * P:(j + 1) * P],
                                 rhs=w_aug[:], start=True, stop=True)
                nc.scalar.copy(hb[:, j * HAUG:(j + 1) * HAUG], ps[:])
            nc.sync.dma_start(
                hA_d[i * PROJ_G * P:(i + 1) * PROJ_G * P, :].rearrange(
                    "(j p) c -> p j c", p=P),
                hb[:].rearrange("p (j c) -> p j c", c=HAUG))

        tc.strict_bb_all_engine_barrier()

        # ---- Phase B: edge processing, For_i over groups of G blocks ----
        MW = HD + HEADS   # msg width 68
        pbase = nc.sync.partition_id() * npc   # this core's global node base
        import contextlib
        rep_cm = tc.For_i(0, repeat, 1) if repeat > 1 else contextlib.nullcontext()
        with rep_cm, tc.For_i(0, nb, G) as b0:
            col0 = b0 * tmax
            row0 = b0 * P

            src_st = stpool.tile([P, G * tmax], I32, tag="src_st")
            nc.sync.dma_start(src_st[:], src_d[:, ds(col0, G * tmax)])
            row_st = stpool.tile([P, G * tmax], F32, tag="row_st")
            nc.sync.dma_start(row_st[:], row_d[:, ds(col0, G * tmax)])

            for g in range(G):
                c0 = g * tmax
                hsrc = epool.tile([P, tmax * HAUG], F32, tag="hsrc")
                for t in range(tmax):
                    nc.gpsimd.indirect_dma_start(
                        out=hsrc[:, t * HAUG:(t + 1) * HAUG],
                        out_offset=None,
                        in_=hA_d[:],
                        in_offset=bass.IndirectOffsetOnAxis(
                            ap=src_st[:, c0 + t:c0 + t + 1], axis=0),
                    )
                # er for the block's 128 nodes, h-major [4, 128], broadcast
                # to every partition via a K=1 matmul.
                er_row = spool.tile([1, HEADS * P], F32, tag="er_row")
                nc.sync.dma_start(
                    er_row[:].rearrange("q (h n) -> q h n", n=P),
                    hA_d[ds(pbase + row0 + g * P, P), HD + HEADS:HAUG]
                    .unsqueeze(0).rearrange("q n h -> q h n"))
                bps = psR.tile([P, HEADS * P], F32)
                nc.tensor.matmul(out=bps[:], lhsT=ones_row[:], rhs=er_row[:],
                                 start=True, stop=True)
                er_all = epool.tile([P, HEADS * P], F32, tag="er_all")
                nc.vector.tensor_copy(er_all[:], bps[:])

                mask = epool.tile([P, tmax * P], F32, tag="mask")
                nc.vector.tensor_tensor(
                    out=mask[:].rearrange("p (t n) -> p t n", n=P),
                    in0=iota_f[:].unsqueeze(1).broadcast_to([P, tmax, P]),
                    in1=row_st[:, c0:c0 + tmax].unsqueeze(2).broadcast_to(
                        [P, tmax, P]),
                    op=mybir.AluOpType.is_equal)

                erd = epool.tile([P, tmax * HEADS], F32, tag="erd")
                prod = epool.tile([P, HEADS * P], F32, tag="prod")
                for t in range(tmax):
                    nc.vector.tensor_tensor(
                        out=prod[:].rearrange("p (h n) -> p h n", n=P),
                        in0=mask[:, t * P:(t + 1) * P].unsqueeze(1)
                            .broadcast_to([P, HEADS, P]),
                        in1=er_all[:].rearrange("p (h n) -> p h n", n=P),
                        op=mybir.AluOpType.mult)
                    nc.vector.tensor_reduce(
                        out=erd[:, t * HEADS:(t + 1) * HEADS],
                        in_=prod[:].rearrange("p (h n) -> p h n", n=P),
                        axis=mybir.AxisListType.X, op=mybir.AluOpType.add)

                hsrc3 = hsrc[:].rearrange("p (t c) -> p t c", c=HAUG)

                lg = spool.tile([P, tmax * HEADS], F32, tag="lg")
                lg3 = lg[:].rearrange("p (t h) -> p t h", h=HEADS)
                nc.vector.tensor_tensor(
                    out=lg3, in0=hsrc3[:, :, HD:HD + HEADS],
                    in1=erd[:].rearrange("p (t h) -> p t h", h=HEADS),
                    op=mybir.AluOpType.add)
                lk = spool.tile([P, tmax * HEADS], F32, tag="lk")
                nc.vector.tensor_scalar_mul(lk[:], lg[:], 0.2)
                nc.vector.tensor_tensor(out=lk[:], in0=lk[:], in1=lg[:],
                                        op=mybir.AluOpType.max)

                msg = epool.tile([P, tmax * MW], F32, tag="msg")
                msg3 = msg[:].rearrange("p (t c) -> p t c", c=MW)
                nc.scalar.activation(out=msg3[:, :, HD:MW], in_=lk[:],
                                     func=mybir.ActivationFunctionType.Exp)
                nc.vector.tensor_tensor(
                    out=msg3[:, :, 0:HD].rearrange("p t (h d) -> p t h d", d=D_OUT),
                    in0=hsrc3[:, :, 0:HD].rearrange("p t (h d) -> p t h d", d=D_OUT),
                    in1=msg3[:, :, HD:MW].unsqueeze(3).broadcast_to(
                        [P, tmax, HEADS, D_OUT]),
                    op=mybir.AluOpType.mult)

                ps = psB.tile([P, MW], F32)
                for t in range(tmax):
                    nc.tensor.matmul(out=ps[:],
                                     lhsT=mask[:, t * P:(t + 1) * P],
                                     rhs=msg[:, t * MW:(t + 1) * MW],
                                     start=(t == 0), stop=(t == tmax - 1))

                den = spool.tile([P, HEADS], F32, tag="den")
                nc.vector.tensor_scalar_max(den[:], ps[:, HD:MW], den_eps)
                rec = spool.tile([P, HEADS], F32, tag="rec")
                nc.vector.reciprocal(rec[:], den[:])

                ob = spool.tile([P, HD], F32, tag="ob")
                nc.vector.tensor_tensor(
                    out=ob[:].rearrange("p (h d) -> p h d", d=D_OUT),
                    in0=ps[:, 0:HD].rearrange("p (h d) -> p h d", d=D_OUT),
                    in1=rec[:].unsqueeze(2).broadcast_to([P, HEADS, D_OUT]),
                    op=mybir.AluOpType.mult)
                nc.vector.tensor_tensor(out=ob[:], in0=ob[:], in1=bias_b[:],
                                        op=mybir.AluOpType.add)
                nc.sync.dma_start(out_d[ds(row0 + g * P, P), :], ob[:])

    nc.compile()
    return nc


_PROGRAM_CACHE = {}


def run(feat, W, attn_l, attn_r, bias, src, dst, n_cores=N_CORES, nb=NB):
    global LAST_RESULTS
    feat = np.asarray(feat, dtype=np.float32)
    src = np.asarray(src, dtype=np.int32)
    dst = np.asarray(dst, dtype=np.int32)

    host = build_host_data(feat, src, dst, n_cores=n_cores, nb=nb)
    tmax = host["tmax"]

    key = (tmax, nb, feat.shape[1], n_cores)
    if key not in _PROGRAM_CACHE:
        _PROGRAM_CACHE[key] = build_program(tmax, nb=nb, in_dim=feat.shape[1],
                                            n_cores=n_cores)
    nc = _PROGRAM_CACHE[key]

    in_maps = []
    for c in range(n_cores):
        in_maps.append({
            "featT": host["featT"],
            "W": np.asarray(W, dtype=np.float32),
            "attn_l": np.asarray(attn_l, dtype=np.float32),
            "attn_r": np.asarray(attn_r, dtype=np.float32),
            "bias": np.asarray(bias, dtype=np.float32),
            "srcs": host["srcs"][c],
            "rows": host["rows"][c],
        })

    res = run_bass_kernel_spmd(nc, in_maps, core_ids=list(range(n_cores)))
    LAST_RESULTS = res
    out = np.concatenate([res.results[c]["out"] for c in range(n_cores)], axis=0)
    return out[host["perm"]]


def kernel(feat, W, attn_l, attn_r, bias, src, dst):
    out = run(feat, W, attn_l, attn_r, bias, src, dst)
    return out[:N_NODES].reshape(N_NODES, HEADS, D_OUT).astype(np.float32)

